# revision 3
# baseline (speedup 1.0000x reference)
"""Trainium2 Bass kernel for ConvertedLlamaAttention (LoRA q/k/v + RoPE + causal attention + out-proj).

Strategy: tensor-parallel over heads across 8 NeuronCores (4 heads/core).
All device matmuls run in "transposed" layouts so no on-device transposes are
needed anywhere:
  - Q^T, K^T computed as W^T-stationary matmuls (head_dim on partitions),
  - V computed in natural layout (seq on partitions) from the same X^T tiles,
  - scores computed transposed (S^T = K^T^T-slices @ Q^T) so softmax sums are
    done with a ones-vector matmul, and A·V consumes V in natural layout,
  - out-proj consumes A·V^T directly as the stationary operand.
LoRA (incl. the half-interleave) is folded into the weights on the host.
Each core emits a partial (2048, 4096) output (row-parallel Wo); the host sums.
"""
import sys

for _p in ("/opt/trn_rl_repo", "/root/.axon_site/_ro/trn_rl_repo"):
    if _p not in sys.path:
        sys.path.insert(0, _p)

import numpy as np
import ml_dtypes

import concourse.bass as bass  # noqa: F401  (registers types)
import concourse.mybir as mybir
import concourse.tile as tile
from concourse import bacc, bass_utils

F32 = mybir.dt.float32
F32R = mybir.dt.float32r
BF16 = mybir.dt.bfloat16

H = 4096          # hidden
S = 2048          # sequence
P = 128           # partitions
HD = 128          # head dim
NCORES = 8
HPC = 4           # heads per core
CW = HPC * HD     # per-core width of q/k/v/attn dims = 512
NCHUNKS = 4       # seq chunks of 512
KCH = H // P      # 32 hidden chunks
LORA_SCALING = 2.0
EXP_SCALE = float(1.0 / np.sqrt(HD))

_CACHE = {}


def _build():
    nc = bacc.Bacc("TRN2", target_bir_lowering=False, debug=False, num_devices=NCORES)

    xt_d = nc.declare_dram_parameter("xt", [H, S], BF16, isOutput=False)
    wq_d = nc.declare_dram_parameter("wq", [H, CW], BF16, isOutput=False)
    wk_d = nc.declare_dram_parameter("wk", [H, CW], BF16, isOutput=False)
    wv_d = nc.declare_dram_parameter("wv", [H, CW], BF16, isOutput=False)
    wot_d = nc.declare_dram_parameter("wot", [CW, H], F32R, isOutput=False)
    cs_d = nc.declare_dram_parameter("cs", [P, S], F32, isOutput=False)
    masks_d = nc.declare_dram_parameter("masks", [P, 4, 512], F32, isOutput=False)
    ones_d = nc.declare_dram_parameter("ones", [P, P], F32R, isOutput=False)
    out_d = nc.declare_dram_parameter("out", [S, H], F32, isOutput=True)

    xt3 = xt_d.rearrange("(ko p) s -> p ko s", p=P)      # (128, 32, 2048)
    wq3 = wq_d.rearrange("(ko p) m -> p ko m", p=P)      # (128, 32, 512)
    wk3 = wk_d.rearrange("(ko p) m -> p ko m", p=P)
    wv3 = wv_d.rearrange("(ko p) m -> p ko m", p=P)
    wot3 = wot_d.rearrange("(h p) n -> p h n", p=P)      # (128, 4, 4096)

    with tile.TileContext(nc) as tc:
        with tc.tile_pool(name="persist", bufs=1) as pp:
            qt = [pp.tile([P, S], F32R, tag=f"qt{h}", name=f"qt{h}") for h in range(HPC)]
            kt = [pp.tile([P, S], F32R, tag=f"kt{h}", name=f"kt{h}") for h in range(HPC)]
            v_sb = pp.tile([P, S // P, CW], F32R, tag="v")   # (128, 16, 512)
            cs_sb = pp.tile([P, S], F32, tag="cs")
            masks_sb = pp.tile([P, 4, 512], F32, tag="masks")
            ones_sb = pp.tile([P, P], F32R, tag="ones")
            nc.sync.dma_start(cs_sb[:], cs_d[:])
            nc.sync.dma_start(masks_sb[:], masks_d[:])
            nc.sync.dma_start(ones_sb[:], ones_d[:])
            ones_col = ones_sb[:, 0:1]
            ones_row = ones_sb[0:1, :]

            def rope(qp, dest, ncx):
                sl = slice(ncx * 512, (ncx + 1) * 512)
                t1 = ropep.tile([P, 512], F32, tag="r1")
                t2 = ropep.tile([P, 512], F32, tag="r2")
                # dest[0:64]  = q1*cos - q2*sin ; dest[64:] = q1*sin + q2*cos
                nc.vector.tensor_mul(t1[0:64], qp[0:64], cs_sb[0:64, sl])
                nc.vector.tensor_mul(t2[0:64], qp[64:128], cs_sb[64:128, sl])
                nc.vector.tensor_sub(dest[0:64], t1[0:64], t2[0:64])
                nc.vector.tensor_mul(t1[64:128], qp[0:64], cs_sb[64:128, sl])
                nc.vector.tensor_mul(t2[64:128], qp[64:128], cs_sb[0:64, sl])
                nc.vector.tensor_add(dest[64:128], t1[64:128], t2[64:128])

            # ---------------- Phase 1: Q^T, K^T, V projections ----------------
            with tc.tile_pool(name="xtp", bufs=5) as xtp, \
                 tc.tile_pool(name="wp", bufs=4) as wp, \
                 tc.tile_pool(name="ropep", bufs=4) as ropep, \
                 tc.tile_pool(name="projps", bufs=8, space="PSUM") as projps:
                for ncx in range(NCHUNKS):
                    ssl = slice(ncx * 512, (ncx + 1) * 512)
                    xts = []
                    for b in range(4):
                        t = xtp.tile([P, 8, 512], BF16, tag="xt")
                        nc.sync.dma_start(t[:], xt3[:, b * 8:(b + 1) * 8, ssl])
                        xts.append(t)

                    qk_ps = [projps.tile([P, 512], F32, tag="proj", name=f"qk_ps{ncx}_{i}") for i in range(8)]
                    for k in range(KCH):
                        wq_t = wp.tile([P, CW], BF16, tag="wq")
                        wk_t = wp.tile([P, CW], BF16, tag="wk")
                        nc.sync.dma_start(wq_t[:], wq3[:, k, :])
                        nc.sync.dma_start(wk_t[:], wk3[:, k, :])
                        rhs = xts[k // 8][:, k % 8, :]
                        for m in range(HPC):
                            nc.tensor.matmul(qk_ps[m][:], lhsT=wq_t[:, m * HD:(m + 1) * HD],
                                             rhs=rhs, start=(k == 0), stop=(k == KCH - 1))
                        for m in range(HPC):
                            nc.tensor.matmul(qk_ps[4 + m][:], lhsT=wk_t[:, m * HD:(m + 1) * HD],
                                             rhs=rhs, start=(k == 0), stop=(k == KCH - 1))
                    for m in range(HPC):
                        rope(qk_ps[m], qt[m][:, ssl], ncx)
                    for m in range(HPC):
                        rope(qk_ps[4 + m], kt[m][:, ssl], ncx)

                    v_ps = [projps.tile([P, 512], F32, tag="proj", name=f"v_ps{ncx}_{i}") for i in range(4)]
                    for k in range(KCH):
                        wv_t = wp.tile([P, CW], BF16, tag="wv")
                        nc.sync.dma_start(wv_t[:], wv3[:, k, :])
                        for t in range(4):
                            nc.tensor.matmul(v_ps[t][:],
                                             lhsT=xts[k // 8][:, k % 8, t * P:(t + 1) * P],
                                             rhs=wv_t[:], start=(k == 0), stop=(k == KCH - 1))
                    for t in range(4):
                        nc.any.tensor_copy(v_sb[:, ncx * 4 + t, :], v_ps[t][:])

            # ---------------- Phase 2: attention + out-proj ----------------
            with tc.tile_pool(name="probsp", bufs=6) as probsp, \
                 tc.tile_pool(name="avtsp", bufs=6) as avtsp, \
                 tc.tile_pool(name="rbp", bufs=2) as rbp, \
                 tc.tile_pool(name="recp", bufs=2) as recp, \
                 tc.tile_pool(name="osbp", bufs=4) as osbp, \
                 tc.tile_pool(name="wotp", bufs=8) as wotp, \
                 tc.tile_pool(name="stps", bufs=2, space="PSUM") as stps, \
                 tc.tile_pool(name="avtps", bufs=2, space="PSUM") as avtps, \
                 tc.tile_pool(name="smallps", bufs=2, space="PSUM") as smallps, \
                 tc.tile_pool(name="outps", bufs=2, space="PSUM") as outps:
                for qc in range(NCHUNKS):
                    qsl = slice(qc * 512, (qc + 1) * 512)
                    avt_sb = []
                    for h in range(HPC):
                        avt_ps = avtps.tile([P, 512], F32, tag="avt")
                        sums_ps = smallps.tile([1, 512], F32, tag="small")
                        nkt = 4 * (qc + 1)
                        for kti in range(nkt):
                            st = stps.tile([P, 512], F32, tag="st")
                            nc.tensor.matmul(st[:], lhsT=kt[h][:, kti * P:(kti + 1) * P],
                                             rhs=qt[h][:, qsl], start=True, stop=True)
                            probs = probsp.tile([P, 512], F32R, tag="probs")
                            nc.scalar.activation(probs[:], st[:],
                                                 mybir.ActivationFunctionType.Exp,
                                                 scale=EXP_SCALE)
                            j = kti - 4 * qc
                            if j >= 0:
                                nc.vector.tensor_mul(probs[:], probs[:], masks_sb[:, j, :])
                            nc.tensor.matmul(avt_ps[:], lhsT=v_sb[:, kti, h * HD:(h + 1) * HD],
                                             rhs=probs[:], start=(kti == 0), stop=(kti == nkt - 1))
                            nc.tensor.matmul(sums_ps[:], lhsT=ones_col,
                                             rhs=probs[:], start=(kti == 0), stop=(kti == nkt - 1))
                        recip = recp.tile([1, 512], F32R, tag="recip")
                        with nc.allow_low_precision(reason="softmax reciprocal in f32r"):
                            nc.vector.reciprocal(recip[:], sums_ps[:])
                        rb_ps = smallps.tile([P, 512], F32, tag="small")
                        nc.tensor.matmul(rb_ps[:], lhsT=ones_row, rhs=recip[:],
                                         start=True, stop=True)
                        rb_sb = rbp.tile([P, 512], F32, tag="rb")
                        nc.any.tensor_copy(rb_sb[:], rb_ps[:])
                        avs = avtsp.tile([P, 512], F32R, tag="avts")
                        nc.vector.tensor_mul(avs[:], avt_ps[:], rb_sb[:])
                        avt_sb.append(avs)
                    for hc in range(8):
                        wts = []
                        for h in range(HPC):
                            wt = wotp.tile([P, 512], F32R, tag="wot", name=f"wot{qc}_{hc}_{h}")
                            nc.sync.dma_start(wt[:], wot3[:, h, hc * 512:(hc + 1) * 512])
                            wts.append(wt)
                        for qs in range(4):
                            o_ps = outps.tile([P, 512], F32, tag="o")
                            for h in range(HPC):
                                nc.tensor.matmul(o_ps[:],
                                                 lhsT=avt_sb[h][:, qs * P:(qs + 1) * P],
                                                 rhs=wts[h][:],
                                                 start=(h == 0), stop=(h == HPC - 1))
                            o_sb = osbp.tile([P, 512], F32, tag="osb")
                            nc.any.tensor_copy(o_sb[:], o_ps[:])
                            nc.sync.dma_start(
                                out_d[qc * 512 + qs * P: qc * 512 + (qs + 1) * P,
                                      hc * 512:(hc + 1) * 512],
                                o_sb[:])

    nc.compile()
    return nc


def _fold(W, A, B):
    """Fold LoRA + its half/interleave permutation into the base weight."""
    BA = (B.astype(np.float64) @ A.astype(np.float64)) * LORA_SCALING
    j = np.arange(H)
    g = np.where(j < H // 2, 2 * j, 2 * (j - H // 2) + 1)
    return (W.astype(np.float64) + BA[g, :]).astype(np.float32)


def _host_consts():
    inv_freq = (1.0 / (10000.0 ** (np.arange(0, HD, 2, dtype=np.float32) / HD))).astype(np.float32)
    freqs = np.arange(S, dtype=np.float32)[:, None] * inv_freq[None, :]   # (S, 64)
    cs = np.concatenate([np.cos(freqs).T, np.sin(freqs).T], axis=0).astype(np.float32)  # (128, S)
    p = np.arange(P)[:, None, None]
    jj = np.arange(4)[None, :, None]
    f = np.arange(512)[None, None, :]
    masks = (jj * P + p <= f).astype(np.float32)          # (128, 4, 512)
    ones = np.ones((P, P), dtype=np.float32)
    return cs, masks, ones


def kernel(hidden_states, Wq, Wk, Wv, Wo, Aq, Bq, Ak, Bk, Av, Bv):
    if "nc" not in _CACHE:
        _CACHE["nc"] = _build()
    nc = _CACHE["nc"]

    x = np.ascontiguousarray(np.asarray(hidden_states, dtype=np.float32)[0])  # (S, H)
    xt_bf = np.ascontiguousarray(x.T).astype(ml_dtypes.bfloat16)

    Wq_eff = _fold(np.asarray(Wq), np.asarray(Aq), np.asarray(Bq))
    Wk_eff = _fold(np.asarray(Wk), np.asarray(Ak), np.asarray(Bk))
    Wv_eff = _fold(np.asarray(Wv), np.asarray(Av), np.asarray(Bv))
    Wo_np = np.asarray(Wo, dtype=np.float32)

    cs, masks, ones = _host_consts()

    in_maps = []
    for c in range(NCORES):
        cols = slice(CW * c, CW * (c + 1))
        in_maps.append({
            "xt": xt_bf,
            "wq": np.ascontiguousarray(Wq_eff[cols].T).astype(ml_dtypes.bfloat16),
            "wk": np.ascontiguousarray(Wk_eff[cols].T).astype(ml_dtypes.bfloat16),
            "wv": np.ascontiguousarray(Wv_eff[cols].T).astype(ml_dtypes.bfloat16),
            "wot": np.ascontiguousarray(Wo_np[:, cols].T),
            "cs": cs,
            "masks": masks,
            "ones": ones,
        })

    res = bass_utils.run_bass_kernel_spmd(nc, in_maps, core_ids=list(range(NCORES)))
    acc = np.zeros((S, H), dtype=np.float64)
    for c in range(NCORES):
        acc += res.results[c]["out"].astype(np.float64)
    return acc.astype(np.float32)[None]
